# revision 11
# baseline (speedup 1.0000x reference)
"""Trainium2 Bass kernel for ChannelAttention (B=32, L=2048, C=1280, H=160).

Reference computation (per batch b):
    gap = mean(x[b], axis=0)            # [C], pool over seq
    gmp = max(x[b], axis=0)             # [C]
    attn = sigmoid(mlp(gap) + mlp(gmp)) # mlp: relu(v@w1+b1)@w2+b2
    y[b] = x[b] * attn[None, :]

Sharding: data-parallel over batch, 4 batches per core on 8 cores.
Each core keeps one batch (10.5MB) resident in SBUF so x is read from
HBM exactly once (read 10.5MB + write 10.5MB per batch).

Per-batch on-core schedule:
  - stream 16 tiles [128, 1280] (seq on partitions, channels on free)
  - channel sums via PE matmul with all-ones stationary [128,1] -> psum [1,1280]
  - channel max via DVE running tensor_max + log2(128) partition fold
  - tiny MLP: PE-transpose [2,1280]->[128,2] chunks, two matmuls, relu/sigmoid on ACT
  - attn row broadcast to 128 partitions via PE matmul with ones stationary
  - gate: DVE in-place multiply of resident tiles, DMA out
"""

import numpy as np
from concourse import bacc, mybir
import concourse.bass as bass
import concourse.tile as tile
from concourse.bass_utils import run_bass_kernel_spmd
from concourse.masks import make_identity

B, L, C, H = 32, 2048, 1280, 160
NCORES = 8
BPC = B // NCORES  # batches per core
P = 128
LT = L // P        # 16 seq tiles per batch
CCH = C // P       # 10 channel chunks
F32 = mybir.dt.float32
NSPLIT = [(0, 512), (512, 1024), (1024, 1280)]  # psum-bank-sized free chunks

_CACHE = {}


def build_nc(nrep=1):
    nc = bacc.Bacc(
        "TRN2",
        target_bir_lowering=False,
        debug=False,
        enable_asserts=False,
        num_devices=NCORES,
    )
    x_d = nc.dram_tensor("x", [BPC, L, C], F32, kind="ExternalInput").ap()
    w1_d = nc.dram_tensor("w1", [C, H], F32, kind="ExternalInput").ap()
    b1_d = nc.dram_tensor("b1", [H], F32, kind="ExternalInput").ap()
    w2_d = nc.dram_tensor("w2", [H, C], F32, kind="ExternalInput").ap()
    b2_d = nc.dram_tensor("b2", [C], F32, kind="ExternalInput").ap()
    y_d = nc.dram_tensor("y", [BPC, L, C], F32, kind="ExternalOutput").ap()

    x_t = x_d.rearrange("b (n p) c -> b n p c", p=P)
    y_t = y_d.rearrange("b (n p) c -> b n p c", p=P)
    w1_t = w1_d.rearrange("(n p) h -> n p h", p=P)

    Relu = mybir.ActivationFunctionType.Relu
    Sigmoid = mybir.ActivationFunctionType.Sigmoid

    with tile.TileContext(nc) as tc:
        with (
            tc.tile_pool(name="const", bufs=1) as const,
            tc.tile_pool(name="xres", bufs=20) as xres,
            tc.tile_pool(name="rmaxp", bufs=2) as rmaxp,
            tc.tile_pool(name="vrowp", bufs=2) as vrowp,
            tc.tile_pool(name="vtp", bufs=2) as vtp,
            tc.tile_pool(name="attnp", bufs=2) as attnp,
            tc.tile_pool(name="psum_s", bufs=1, space="PSUM") as psum_s,
            tc.tile_pool(name="psum_m", bufs=1, space="PSUM") as psum_m,
        ):
            # ---- constants / weights (loaded once) ----
            ones_col = const.tile([P, 1], F32, tag="ones_col")
            nc.vector.memset(ones_col[:], 1.0)
            ones_row = const.tile([1, P], F32, tag="ones_row")
            nc.vector.memset(ones_row[:], 1.0)
            ident128 = const.tile([P, P], F32, tag="ident128")
            make_identity(nc, ident128[:])

            w1_sb = []
            for t in range(CCH):
                w1c = const.tile([P, H], F32, tag=f"w1_{t}")
                nc.sync.dma_start(w1c[:], w1_t[t])
                w1_sb.append(w1c)
            w2a = const.tile([P, C], F32, tag="w2a")
            nc.sync.dma_start(w2a[:], w2_d[0:P])
            w2b = const.tile([H - P, C], F32, tag="w2b")
            nc.sync.dma_start(w2b[:], w2_d[P:H])
            b1a = const.tile([P, 1], F32, tag="b1a")
            nc.sync.dma_start(b1a[:], b1_d[0:P, None])
            b1b = const.tile([H - P, 1], F32, tag="b1b")
            nc.sync.dma_start(b1b[:], b1_d[P:H, None])
            b2row = const.tile([1, C], F32, tag="b2row")
            nc.sync.dma_start(b2row[:], b2_d[None, :])
            b2x2 = const.tile([1, C], F32, tag="b2x2")
            nc.vector.tensor_scalar_mul(b2x2[:], b2row[:], 2.0)

            for rep, b in [(r, b) for r in range(nrep) for b in range(BPC)]:
                # ---- pass 1: stream tiles, accumulate channel sum + max ----
                s = psum_s.tile([1, C], F32, tag="s")
                rmax = rmaxp.tile([P, C], F32, tag="rmax")
                xts = []
                for t in range(LT):
                    xt = xres.tile([P, C], F32, tag="x", name=f"x_{b}_{t}")
                    nc.sync.dma_start(xt[:], x_t[b, t])
                    xts.append(xt)
                    first, last = t == 0, t == LT - 1
                    for n0, n1 in NSPLIT:
                        nc.tensor.matmul(
                            s[0:1, n0:n1], ones_col[:], xt[:, n0:n1],
                            start=first, stop=last,
                        )
                    if t == 1:
                        nc.vector.tensor_max(rmax[:], xts[0][:], xts[1][:])
                    elif t > 1:
                        nc.vector.tensor_max(rmax[:], rmax[:], xt[:])

                # ---- finalize stats ----
                # gap row in SBUF (scaled sum); the partition-axis max is
                # finished by transposing each [128, 128] chunk on the PE and
                # reducing along the free axis (HW forbids SB-SB tensor ops
                # with mismatched start partitions, so no partition folds).
                gap_row = vrowp.tile([1, C], F32, tag="gap")
                nc.vector.tensor_scalar_mul(gap_row[0:1, :], s[0:1, :], 1.0 / L)

                # vT[:, 2t] = gap chunk t (C on partitions), vT[:, 2t+1] = gmp
                vT = vtp.tile([P, 2 * CCH], F32, tag="vT")
                for t in range(CCH):
                    tp = psum_m.tile(
                        [P, 512], F32, tag="tpbc", bufs=2, name=f"tp_{b}_{t}"
                    )
                    nc.tensor.transpose(
                        tp[:, 0:P], rmax[:, bass.ts(t, P)], ident128[:]
                    )
                    nc.tensor.transpose(
                        tp[:, P : P + 1], gap_row[0:1, bass.ts(t, P)],
                        ones_row[0:1, 0:1],
                    )
                    nc.vector.tensor_copy(vT[:, 2 * t : 2 * t + 1], tp[:, P : P + 1])
                    nc.vector.reduce_max(
                        vT[:, 2 * t + 1 : 2 * t + 2], tp[:, 0:P],
                        axis=mybir.AxisListType.X,
                    )

                # ---- mm1: hT = w1^T @ v  (H on partitions, 2 cols) ----
                hh1 = psum_m.tile([P, 2], F32, tag="tpbc", bufs=2, name=f"hh1_{b}")
                hh2 = psum_m.tile(
                    [H - P, 2], F32, tag="tpbc", bufs=2, name=f"hh2_{b}"
                )
                for t in range(CCH):
                    first, last = t == 0, t == CCH - 1
                    nc.tensor.matmul(
                        hh1[:, :], w1_sb[t][:, 0:P], vT[:, 2 * t : 2 * t + 2],
                        start=first, stop=last,
                    )
                    nc.tensor.matmul(
                        hh2[:, :], w1_sb[t][:, P:H], vT[:, 2 * t : 2 * t + 2],
                        start=first, stop=last,
                    )
                hT = vtp.tile([P, 4], F32, tag="hT")
                nc.scalar.activation(hT[:, 0:2], hh1[:, :], Relu, bias=b1a[:])
                nc.scalar.activation(
                    hT[0 : H - P, 2:4], hh2[:, :], Relu, bias=b1b[:]
                )

                # ---- mm2: o = hT_gap^T@w2 + hT_gmp^T@w2 + 2*b2  -> [1, C] ----
                o = psum_m.tile([1, C], F32, tag="o")
                for n0, n1 in NSPLIT:
                    nc.tensor.matmul(o[0:1, n0:n1], hT[:, 0:1], w2a[:, n0:n1],
                                     start=True, stop=False)
                    nc.tensor.matmul(o[0:1, n0:n1], hT[0 : H - P, 2:3],
                                     w2b[:, n0:n1], start=False, stop=False)
                    nc.tensor.matmul(o[0:1, n0:n1], hT[:, 1:2], w2a[:, n0:n1],
                                     start=False, stop=False)
                    nc.tensor.matmul(o[0:1, n0:n1], hT[0 : H - P, 3:4],
                                     w2b[:, n0:n1], start=False, stop=False)
                    nc.tensor.matmul(o[0:1, n0:n1], ones_row[0:1, 0:1],
                                     b2x2[0:1, n0:n1], start=False, stop=True)
                attn_row = vrowp.tile([1, C], F32, tag="arow")
                nc.scalar.activation(attn_row[:], o[0:1, :], Sigmoid)

                # ---- broadcast attn row across 128 partitions ----
                attn = attnp.tile([P, C], F32, tag="attn")
                for n0, n1 in NSPLIT:
                    bc = psum_m.tile(
                        [P, 512], F32, tag="tpbc", bufs=2, name=f"bc_{b}_{n0}"
                    )
                    nc.tensor.matmul(
                        bc[:, 0 : n1 - n0], ones_row[:], attn_row[0:1, n0:n1],
                        start=True, stop=True,
                    )
                    nc.vector.tensor_copy(attn[:, n0:n1], bc[:, 0 : n1 - n0])

                # ---- pass 2: gate resident tiles in place, store ----
                for t in range(LT):
                    nc.vector.tensor_mul(xts[t][:], xts[t][:], attn[:])
                    nc.sync.dma_start(y_t[b, t], xts[t][:])

    nc.compile()
    return nc


def get_nc():
    if "nc" not in _CACHE:
        _CACHE["nc"] = build_nc()
    return _CACHE["nc"]


def kernel(x, w1, b1, w2, b2):
    x = np.ascontiguousarray(np.asarray(x, dtype=np.float32))
    w1 = np.ascontiguousarray(np.asarray(w1, dtype=np.float32))
    b1 = np.ascontiguousarray(np.asarray(b1, dtype=np.float32))
    w2 = np.ascontiguousarray(np.asarray(w2, dtype=np.float32))
    b2 = np.ascontiguousarray(np.asarray(b2, dtype=np.float32))
    nc = get_nc()
    in_maps = [
        {
            "x": x[c * BPC : (c + 1) * BPC],
            "w1": w1,
            "b1": b1,
            "w2": w2,
            "b2": b2,
        }
        for c in range(NCORES)
    ]
    res = run_bass_kernel_spmd(nc, in_maps, core_ids=list(range(NCORES)))
    return np.concatenate([res.results[c]["y"] for c in range(NCORES)], axis=0)


if __name__ == "__main__":
    nc = build_nc()
    print("build + compile OK")
